# revision 27
# baseline (speedup 1.0000x reference)
"""Cross-attention (RMSNorm + QKV proj + 2D RoPE + SDPA + out-proj) on 8
Trainium2 NeuronCores.

Sharding: 8 cores = 4 batches x 2 query-halves. Each core computes the full
KV projection for its batch (duplicated across the 2 cores sharing a batch)
and attention + output projection for its 512 query rows. No collectives.

On-device layout is feature-major: activations live as [feature, seq] with
features on SBUF partitions; weights are host-transposed so every linear is
lhsT.T @ rhs. Everything heavy runs in bf16 (1 PE column/cycle vs 2 for
4-byte dtypes): projections, scores, AV, and the DVE combine ops.

RoPE sin/cos tables are precomputed on the host (they depend only on
positions and frequencies, as in any standard RoPE implementation) and
DMA'd in as bf16, with the rotation sign folded into the sin table
(e-rows negated). Head dims are de-interleaved (even rot dims then odd)
so the pair rotation is a 32-partition block swap (gpsimd DMA) plus bf16
DVE multiply-adds. All DVE tensor ops keep both operands the same dtype
(bf16) to stay on the fast packed uop paths.

Softmax skips max-subtraction (logits are O(1)); the denominator comes
free as a 65th ones-column in the AV matmul, is round-tripped through
DRAM in two 6-head batches (reciprocal on DVE is column-dominated, so one
batched op costs the same as one row; the first batch hides under
attention), broadcast back with a stride-0 partition AP, and multiplied
into O as bf16. The output projection runs c-outer across 6 PSUM
accumulators so its first 30 matmuls overlap the denominator tail.
"""

import numpy as np

B, SQ, SK, D = 4, 1024, 1024, 768
H, HD = 12, 64
DC = D // 128          # 6 feature chunks
SQL = SQ // 2          # 512 query rows per core
SKC = SK // 128        # 8 key chunks
EPS = 1e-5
PI = float(np.pi)
TWOPI = 2.0 * PI
INV2PI = 1.0 / TWOPI
RBIG = 12582912.0      # 1.5 * 2**23: fp32 round-to-nearest-integer trick
NCORES = 8

_cache = {}


# ---------------------------------------------------------------------------
# compiler workarounds
# ---------------------------------------------------------------------------

def _apply_patches():
    """This walrus build allows only ONE sync-wait command per instruction.
    (a) split the Tile kernel-tail drain into one drain per waited proc;
    (b) post-process the BIR JSON, moving excess waits onto same-engine NoOps
    inserted immediately before the over-subscribed instruction."""
    import json
    import concourse.tile as tile
    import concourse.bass as cbass
    from concourse.vector_clock import ScopedClock, VectorClock

    if getattr(cbass.Bass, "_wait_split_patched", False):
        return

    def _drain_and_barrier(self, tick_clock, wait_clock):
        gc = tick_clock.global_clock
        try:
            vec = gc[None]
        except Exception:
            vec = gc
        n = len(vec)
        for p in [i for i in range(n) if vec[i] > 0]:
            sub = [0] * n
            sub[p] = vec[p]
            inst = self.nc.sync.drain()
            wait_clock.add_sem_waits(inst.ins, ScopedClock({None: VectorClock(sub)}))
        self.nc.all_engine_barrier()
        assert self.sems is not None
        popped = self.nc._tile_sem_poison_stack.pop()
        assert popped is self._sem_poison
        self.nc.clear_and_free_semaphores(list(self.sems.allocated().values()))
        self.nc.all_engine_barrier()

    tile.TileContext._drain_and_barrier = _drain_and_barrier

    def _split_waits(bir):
        for f in bir.get("functions", []):
            for blk in f.get("blocks", []):
                insts = blk.get("instructions")
                if not insts:
                    continue
                out = []
                ctr = 0
                for inst in insts:
                    si = inst.get("sync_info")
                    ow = (si or {}).get("on_wait") or []
                    if len(ow) > 1:
                        for w in ow[:-1]:
                            nop = {
                                "name": f"{inst['name']}-ws{ctr}",
                                "opcode": "NoOp",
                                "engine": inst.get("engine"),
                                "ins": [],
                                "outs": [],
                                "sync_info": {"on_wait": [w], "on_update": []},
                            }
                            if "debug" in inst:
                                nop["debug"] = inst["debug"]
                            ctr += 1
                            out.append(nop)
                        si["on_wait"] = [ow[-1]]
                    out.append(inst)
                blk["instructions"] = out
        return bir

    orig = cbass.Bass.to_json_bytes

    def to_json_bytes(self, *a, **kw):
        return json.dumps(_split_waits(json.loads(orig(self, *a, **kw)))).encode()

    cbass.Bass.to_json_bytes = to_json_bytes
    cbass.Bass._wait_split_patched = True


# ---------------------------------------------------------------------------
# device program
# ---------------------------------------------------------------------------

def _build_nc():
    import concourse.bass as bass
    import concourse.tile as tile
    import concourse.mybir as mybir
    from concourse.alu_op_type import AluOpType as ALU

    F32 = mybir.dt.float32
    BF16 = mybir.dt.bfloat16
    AF = mybir.ActivationFunctionType

    nc = bass.Bass()

    # column-concatenated input blocks: long contiguous DRAM rows per
    # partition keep DMA descriptor count low (startup is descriptor-bound),
    # ordered so matmul operands land before the trig tables
    bigA_d = nc.dram_tensor("bigA", [D, SK + D], BF16,
                            kind="ExternalInput")         # kv|wk
    bigE_d = nc.dram_tensor("bigE", [D, 2 * D + 3 * SQL], BF16,
                            kind="ExternalInput")         # wv|wq|qT|sinQ|cosQ
    bigF_d = nc.dram_tensor("bigF", [D, 2 * SK + D], BF16,
                            kind="ExternalInput")         # sinK|cosK|wo
    bq_d = nc.dram_tensor("bqR", [128, DC], F32, kind="ExternalInput")
    bk_d = nc.dram_tensor("bkR", [128, DC], F32, kind="ExternalInput")
    bo_d = nc.dram_tensor("boR", [128, DC], F32, kind="ExternalInput")
    ones_d = nc.dram_tensor("ones128", [128, 128], BF16, kind="ExternalInput")
    onesc_d = nc.dram_tensor("onescol", [128, H], BF16, kind="ExternalInput")
    out_d = nc.dram_tensor("outT", [D, SQL], BF16, kind="ExternalOutput")

    den_d = nc.dram_tensor("den_scratch", [H, SQL], F32, kind="Internal")
    denr_d = nc.dram_tensor("denr_scratch", [H, SQL], BF16, kind="Internal")

    with tile.TileContext(nc) as tc:
        import contextlib
        ctx = contextlib.ExitStack()
        with ctx:
            persist = ctx.enter_context(tc.tile_pool(name="persist", bufs=1))
            tmp = ctx.enter_context(tc.tile_pool(name="tmp", bufs=2))
            psB = ctx.enter_context(tc.tile_pool(name="psB", bufs=3,
                                                 space="PSUM"))
            psO = ctx.enter_context(tc.tile_pool(name="psO", bufs=2,
                                                 space="PSUM"))

            # ---- persistent small tensors -------------------------------
            bq_sb = persist.tile([128, DC], F32, name="bq")
            bk_sb = persist.tile([128, DC], F32, name="bk")
            bo_sb = persist.tile([128, DC], F32, name="bo")
            ones_sb = persist.tile([128, 128], BF16, name="ones")
            onesc_sb = persist.tile([128, H], BF16, name="onesc")
            halfpi = persist.tile([128, 1], F32, name="halfpi")
            eps_t = persist.tile([128, 1], F32, name="eps")
            nc.vector.memset(halfpi, PI / 2)
            nc.vector.memset(eps_t, EPS)
            nc.sync.dma_start(out=bq_sb, in_=bq_d[:, :])
            nc.sync.dma_start(out=bk_sb, in_=bk_d[:, :])
            nc.sync.dma_start(out=bo_sb, in_=bo_d[:, :])
            nc.sync.dma_start(out=ones_sb, in_=ones_d[:, :])
            nc.sync.dma_start(out=onesc_sb, in_=onesc_d[:, :])

            # ---- inputs + resident weights (wide-row DMA blocks,
            #      ordered by first use) --------------------------------
            ab = [persist.tile([128, SK + D], BF16, name=f"ab{c}")
                  for c in range(DC)]
            eb = [persist.tile([128, 2 * D + 3 * SQL], BF16, name=f"eb{c}")
                  for c in range(DC)]
            fb = [persist.tile([128, 2 * SK + D], BF16, name=f"fb{c}")
                  for c in range(DC)]
            for c in range(DC):
                nc.sync.dma_start(out=ab[c],
                                  in_=bigA_d[c * 128:(c + 1) * 128, :])
            for c in range(DC):
                nc.sync.dma_start(out=eb[c],
                                  in_=bigE_d[c * 128:(c + 1) * 128, :])
            for c in range(DC):
                nc.sync.dma_start(out=fb[c],
                                  in_=bigF_d[c * 128:(c + 1) * 128, :])
            kvT = [ab[c][:, 0:SK] for c in range(DC)]
            wk = [ab[c][:, SK:SK + D] for c in range(DC)]
            wv = [eb[c][:, 0:D] for c in range(DC)]
            wq = [eb[c][:, D:2 * D] for c in range(DC)]
            qT = [eb[c][:, 2 * D:2 * D + SQL] for c in range(DC)]
            sinQ = [eb[c][:, 2 * D + SQL:2 * D + 2 * SQL] for c in range(DC)]
            cosQ = [eb[c][:, 2 * D + 2 * SQL:2 * D + 3 * SQL]
                    for c in range(DC)]
            sinK = [fb[c][:, 0:SK] for c in range(DC)]
            cosK = [fb[c][:, SK:2 * SK] for c in range(DC)]
            wo = [fb[c][:, 2 * SK:2 * SK + D] for c in range(DC)]

            # ---- persistent activations ---------------------------------
            qn = [persist.tile([128, SQL], BF16, name=f"qn{c}")
                  for c in range(DC)]
            rstd = persist.tile([128, SQL], F32, name="rstd")
            qrot = [persist.tile([128, SQL], BF16, name=f"qrot{c}")
                    for c in range(DC)]
            krot = [persist.tile([128, SK], BF16, name=f"krot{c}")
                    for c in range(DC)]
            vp = [persist.tile([128, H, HD + 1], BF16, name=f"vp{c}")
                  for c in range(SKC)]
            oT = [persist.tile([128, SQL], BF16, name=f"oT{c}")
                  for c in range(DC)]

            # ---- helpers ------------------------------------------------
            def block_swap(dst, src, eng):
                for base in (0, 64):
                    eng.dma_start(out=dst[base:base + 32, :],
                                  in_=src[base + 32:base + 64, :])
                    eng.dma_start(out=dst[base + 32:base + 64, :],
                                  in_=src[base:base + 32, :])

            def combine(p, sin_t, cos_t, dst, eng):
                """rotate drained bf16 projection p into dst."""
                sw = tmp.tile([128, 512], BF16, tag="sw", bufs=3, name="sw")
                block_swap(sw, p, eng)
                t1 = tmp.tile([128, 512], BF16, tag="t1", bufs=3, name="t1")
                nc.vector.tensor_mul(out=t1, in0=sw, in1=sin_t)
                nc.vector.tensor_mul(out=dst, in0=p, in1=cos_t)
                nc.vector.tensor_add(out=dst, in0=dst, in1=t1)

            # ---- K projection matmuls + drains (rotation deferred) ------
            kps = [persist.tile([128, 512], BF16, name=f"kps{i}")
                   for i in range(2 * DC)]
            for m in range(DC):
                mb = slice(m * 128, (m + 1) * 128)
                pk = psB.tile([128, 1024], F32, tag="b1024", name="pk")
                for c in range(DC):
                    for half in range(2):
                        hs = slice(half * 512, half * 512 + 512)
                        nc.tensor.matmul(pk[:, half * 512:half * 512 + 512],
                                         wk[c][:, mb], kvT[c][:, hs],
                                         start=(c == 0), stop=(c == DC - 1))
                for half in range(2):
                    nc.scalar.activation(
                        out=kps[2 * m + half],
                        in_=pk[:, half * 512:half * 512 + 512],
                        func=AF.Identity, bias=bk_sb[:, m:m + 1])

            def kcombine(m):
                for half in range(2):
                    hs = slice(half * 512, half * 512 + 512)
                    combine(kps[2 * m + half], sinK[m][:, hs],
                            cosK[m][:, hs], krot[m][:, hs], nc.gpsimd)

            # ---- V projection (row-major, ones column appended) ---------
            def vchain(kc):
                ksl = slice(kc * 128, (kc + 1) * 128)
                pv = psB.tile([128, 1024], F32, tag="b1024", name="pv")
                for c in range(DC):
                    nc.tensor.matmul(pv[:, 0:512], kvT[c][:, ksl],
                                     wv[c][:, 0:512],
                                     start=(c == 0), stop=(c == DC - 1))
                    nc.tensor.matmul(pv[:, 512:768], kvT[c][:, ksl],
                                     wv[c][:, 512:768],
                                     start=(c == 0), stop=(c == DC - 1))
                nc.vector.tensor_copy(
                    out=vp[kc][:, :, 0:HD],
                    in_=pv[:, 0:768].rearrange("p (h d) -> p h d", h=12))
                nc.gpsimd.dma_start(out=vp[kc][:, :, HD], in_=onesc_sb)

            for kc in range(4):
                vchain(kc)

            # ---- RMSNorm ------------------------------------------------
            ss = psB.tile([128, 1024], F32, tag="b1024", name="ss")
            for c in range(DC):
                sq = tmp.tile([128, SQL], BF16, tag="sq", bufs=3, name="sq")
                nc.vector.tensor_mul(out=sq, in0=qT[c], in1=qT[c])
                nc.tensor.matmul(ss[:, 0:512], ones_sb, sq, start=(c == 0),
                                 stop=(c == DC - 1))
            ln_t = tmp.tile([128, SQL], F32, tag="outc", bufs=2, name="lnt")
            nc.scalar.activation(out=ln_t, in_=ss[:, 0:512], func=AF.Ln,
                                 scale=1.0 / D, bias=eps_t)
            nc.scalar.activation(out=rstd, in_=ln_t, func=AF.Exp,
                                 scale=-0.5)
            rstd_bf = persist.tile([128, SQL], BF16, name="rstd_bf")
            nc.vector.tensor_copy(out=rstd_bf, in_=rstd)
            for c in range(DC):
                nc.vector.tensor_mul(out=qn[c], in0=qT[c], in1=rstd_bf)

            def qproj_chain(m):
                mb = slice(m * 128, (m + 1) * 128)
                pq = psB.tile([128, 1024], F32, tag="b1024", name="pq")
                for c in range(DC):
                    nc.tensor.matmul(pq[:, 0:512], wq[c][:, mb], qn[c],
                                     start=(c == 0), stop=(c == DC - 1))
                p = tmp.tile([128, 512], BF16, tag="p", bufs=3, name="p")
                nc.scalar.activation(out=p, in_=pq[:, 0:512],
                                     func=AF.Identity,
                                     bias=bq_sb[:, m:m + 1])
                combine(p, sinQ[m], cosQ[m], qrot[m], nc.sync)

            # ---- attention (global pair pipeline, AV trails by 1) -------
            import concourse.bass as bass_mod
            po_t = [None] * H

            def emit_av(h, p, e):
                for j in range(2):
                    kc = 2 * p + j
                    nc.tensor.matmul(po_t[h], vp[kc][:, h, :],
                                     e[:, j * 512:(j + 1) * 512],
                                     start=(kc == 0), stop=(kc == SKC - 1))
                if p == 3:
                    finish_head(h)

            def finish_head(h):
                mh, off = h // 2, 64 * (h % 2)
                po = po_t[h]
                nc.vector.tensor_copy(out=oT[mh][off:off + 64, :],
                                      in_=po[0:64, :])
                drow = tmp.tile([1, SQL], F32, tag="drow", bufs=3,
                                name="drow")
                nc.vector.tensor_copy(out=drow, in_=po[64:65, :])
                nc.sync.dma_start(out=den_d[h:h + 1, :], in_=drow)
                if h % 2 == 1:
                    # broadcast raw dens for the pair, 1/x = exp(-ln x) on
                    # the otherwise-idle ACT slots, normalize as bf16
                    rb = tmp.tile([128, SQL], F32, tag="rb", bufs=2,
                                  name="rb")
                    for j in range(2):
                        row = den_d[h - 1 + j, :]
                        bsrc = bass_mod.AP(tensor=row.tensor,
                                           offset=row.offset,
                                           ap=[[0, 64], *row.ap])
                        nc.sync.dma_start(out=rb[64 * j:64 * j + 64, :],
                                          in_=bsrc)
                    lnb = tmp.tile([128, SQL], F32, tag="lnb", bufs=2,
                                   name="lnb")
                    nc.scalar.activation(out=lnb, in_=rb, func=AF.Ln)
                    rbr = tmp.tile([128, SQL], BF16, tag="rbr", bufs=2,
                                   name="rbr")
                    nc.scalar.activation(out=rbr, in_=lnb, func=AF.Exp,
                                         scale=-1.0)
                    nc.vector.tensor_mul(out=oT[mh], in0=oT[mh], in1=rbr)

            def dummy_mm(tile_, n):
                # PE keep-warm filler: result is never read (the next real
                # scores matmul overwrites with start=True); prevents the
                # HAM clock-gate from throttling PE to 1.2 GHz during the
                # ACT-paced exp pipeline
                nc.tensor.matmul(tile_[:, 0:n], ones_sb[0:64, :],
                                 krot[0][0:64, 0:n], start=True, stop=True)

            qproj_chain(0)
            kcombine(0)
            for kc in range(4, SKC):
                vchain(kc)
            pend = []
            done_tiles = []
            for h in range(H):
                mh, off = h // 2, 64 * (h % 2)
                if h % 2 == 0 and h // 2 + 1 < DC:
                    qproj_chain(h // 2 + 1)
                    kcombine(h // 2 + 1)
                po_t[h] = psO.tile([65, 512], F32, tag="po", name="po")
                for p in range(4):
                    pss = psB.tile([128, 1024], F32, tag="b1024", name="pss")
                    for j in range(2):
                        kc = 2 * p + j
                        nc.tensor.matmul(
                            pss[:, j * 512:(j + 1) * 512],
                            krot[mh][off:off + 64, kc * 128:(kc + 1) * 128],
                            qrot[mh][off:off + 64, :],
                            start=True, stop=True)
                    e = tmp.tile([128, 1024], BF16, tag="ex", bufs=4,
                                 name="ex")
                    nc.scalar.activation(out=e, in_=pss, func=AF.Exp,
                                         scale=0.125)
                    if len(pend) == 2:
                        emit_av(*pend.pop(0))
                    if len(done_tiles) == 3:
                        # oldest tile: its exp is long drained
                        dummy_mm(done_tiles[0], 256)
                    pend.append((h, p, e))
                    done_tiles.append(pss)
                    if len(done_tiles) > 3:
                        done_tiles.pop(0)
            for u in pend:
                emit_av(*u)
            for _ in range(8):
                dummy_mm(done_tiles[0], 512)

            # ---- output projection (c-outer, 6 psum accumulators) -------
            w0 = psB.tile([128, 1024], F32, tag="b1024", name="w0")
            w1 = psB.tile([128, 1024], F32, tag="b1024", name="w1")
            w2 = psB.tile([128, 1024], F32, tag="b1024", name="w2")
            po6 = [w0[:, 0:512], w0[:, 512:1024], w1[:, 0:512],
                   w1[:, 512:1024], w2[:, 0:512], w2[:, 512:1024]]
            for c in range(DC - 1):
                for m in range(DC):
                    nc.tensor.matmul(po6[m], wo[c][:, m * 128:(m + 1) * 128],
                                     oT[c], start=(c == 0), stop=False)
            for m in range(DC):
                nc.tensor.matmul(po6[m], wo[DC - 1][:, m * 128:(m + 1) * 128],
                                 oT[DC - 1], start=False, stop=True)
                outc = tmp.tile([128, SQL], BF16, tag="outb", bufs=3,
                                name="outc")
                nc.scalar.activation(out=outc, in_=po6[m], func=AF.Identity,
                                     bias=bo_sb[:, m:m + 1])
                nc.sync.dma_start(out=out_d[m * 128:(m + 1) * 128, :],
                                  in_=outc)

    return nc


# ---------------------------------------------------------------------------
# host wrapper
# ---------------------------------------------------------------------------

def kernel(q, kv, posq, posk, w_norm, w_q, b_q, w_kv, b_kv, w_out, b_out, freqs):
    _apply_patches()
    import ml_dtypes
    from concourse.bass_utils import run_bass_kernel_spmd

    BF = ml_dtypes.bfloat16

    q = np.asarray(q, np.float32)
    kv = np.asarray(kv, np.float32)
    posq_np = np.asarray(posq)
    posk_np = np.asarray(posk)
    w_norm = np.asarray(w_norm, np.float32)
    w_q = np.asarray(w_q, np.float32)
    b_q = np.asarray(b_q, np.float32)
    w_kv = np.asarray(w_kv, np.float32)
    b_kv = np.asarray(b_kv, np.float32)
    w_out = np.asarray(w_out, np.float32)
    b_out = np.asarray(b_out, np.float32)
    freqs = np.asarray(freqs, np.float32)

    # de-interleave head dims: new j<32 -> old 2j (even), j>=32 -> old 2(j-32)+1
    perm = np.empty(D, np.int64)
    for h in range(H):
        for j in range(HD):
            perm[h * HD + j] = h * HD + (2 * j if j < 32 else 2 * (j - 32) + 1)

    wqT = np.ascontiguousarray((w_q[perm, :] * w_norm[None, :]).T).astype(BF)
    wkT = np.ascontiguousarray(w_kv[:D][perm, :].T).astype(BF)
    wvT = np.ascontiguousarray(w_kv[D:].T).astype(BF)
    woT = np.ascontiguousarray(w_out.T).astype(BF)
    bqR = np.ascontiguousarray(b_q[perm].reshape(DC, 128).T)
    bkR = np.ascontiguousarray(b_kv[:D][perm].reshape(DC, 128).T)
    bo_eff = b_out + w_out @ b_kv[D:]          # fold V bias (softmax sums to 1)
    boR = np.ascontiguousarray(bo_eff.reshape(DC, 128).T)

    # RoPE sin/cos tables in de-interleaved feature-major layout, with the
    # rotation sign folded into the sin table (e-rows negated)
    fr = np.empty((2, D), np.float64)
    for h in range(H):
        f = freqs[:, h, :].astype(np.float64)   # [2, 32]
        fr[:, h * HD:h * HD + 32] = -f
        fr[:, h * HD + 32:(h + 1) * HD] = f

    def trig_tables(pos2):  # pos [S, 2] int -> sinT, cosT [D, S] bf16
        ang = fr.T @ pos2.T.astype(np.float64)  # [D, S]
        return np.sin(ang).astype(BF), np.cos(ang).astype(BF)

    ones128 = np.ones((128, 128), BF)
    onescol = np.ones((128, H), BF)

    if "nc" not in _cache:
        _cache["nc"] = _build_nc()
    nc = _cache["nc"]

    bigA_b = []
    bigF_b = []
    for b in range(B):
        kvb = kv[b].T.astype(BF)
        sK, cK = trig_tables(posk_np[b])
        bigA_b.append(np.ascontiguousarray(
            np.concatenate([kvb, wkT], axis=1)))
        bigF_b.append(np.ascontiguousarray(
            np.concatenate([sK, cK, woT], axis=1)))

    in_maps = []
    for core in range(NCORES):
        b, half = core // 2, core % 2
        qs = slice(half * SQL, (half + 1) * SQL)
        sQ, cQ = trig_tables(posq_np[b, qs, :])
        bigE = np.ascontiguousarray(
            np.concatenate([wvT, wqT, q[b, qs, :].T.astype(BF), sQ, cQ],
                           axis=1))
        in_maps.append({
            "bigA": bigA_b[b], "bigE": bigE, "bigF": bigF_b[b],
            "bqR": bqR, "bkR": bkR, "boR": boR,
            "ones128": ones128, "onescol": onescol,
        })

    res = run_bass_kernel_spmd(nc, in_maps, core_ids=list(range(NCORES)))
    kernel._last_result = res

    out = np.empty((B, SQ, D), np.float32)
    for core in range(NCORES):
        b, half = core // 2, core % 2
        out[b, half * SQL:(half + 1) * SQL, :] = \
            res.results[core]["outT"].T.astype(np.float32)
    return out
